# revision 1
# baseline (speedup 1.0000x reference)
"""RWKV-style Block kernel for 8 Trainium2 NeuronCores (batch-parallel SPMD).

Strategy:
  - Host pre-computes: transposed bf16 weights (with ln affine scale folded in),
    per-channel constants (decay lambda, e^u, mixes, biases, t=0 fixups).
  - Each core processes one batch element b fully on-device:
      ln1 stats -> hn (normalized, bf16) -> DMA-transpose -> hnT
      time-mixes (tensor_scalar ops in transposed land)
      k/v/r matmuls (bf16, PSUM f32)
      WKV via linear recurrence using tensor_tensor_scan (fp32)
      att/short matmuls -> out1 -> ln2 -> gn -> FFN (relu^2 MLP) -> out
  - Everything streamed through SBUF in chunks, large intermediates spilled to
    DRAM scratch (hnT, rwkvT, srT, kk^2, out1).
"""
import numpy as np
import ml_dtypes

import concourse.bass as bass
import concourse.bacc as bacc
import concourse.mybir as mybir
import concourse.tile as tile
from concourse.bass_utils import run_bass_kernel_spmd

F32 = mybir.dt.float32
BF16 = mybir.dt.bfloat16
AL = mybir.AluOpType
ACT = mybir.ActivationFunctionType
BF = ml_dtypes.bfloat16

B, C, F = 8, 1024, 4096
P = 128
CT = C // P          # 8 c-tiles
FT = F // P          # 32 f-tiles
NC2 = C // 512       # 2
EPS = 1e-5

# cvec slots
S_LAM, S_EU, S_MK, S_MV, S_MR, S_KB, S_VB, S_RB, S_FK, S_FV, S_FR, S_FFK, S_FFR, S_RRB, S_FRR = range(15)
NSLOT = 15


def _bcast_free(col_ap, n):
    """per-partition [128,1] column AP -> [128,n] stride-0 broadcast AP."""
    return bass.AP(tensor=col_ap.tensor, offset=col_ap.offset,
                   ap=[col_ap.ap[0], [0, n]])


def build_nc(T):
    NJ = T // 512        # big chunks
    TT = T // 128        # t-tiles
    nc = bacc.Bacc("TRN2", target_bir_lowering=False)

    # ---------------- DRAM I/O ----------------
    x_d = nc.dram_tensor("x", [T, C], F32, kind="ExternalInput")
    wkT_d = nc.dram_tensor("wkT", [C, C], BF16, kind="ExternalInput")
    wvT_d = nc.dram_tensor("wvT", [C, C], BF16, kind="ExternalInput")
    wrT_d = nc.dram_tensor("wrT", [C, C], BF16, kind="ExternalInput")
    woT_d = nc.dram_tensor("woT", [C, C], BF16, kind="ExternalInput")
    shT_d = nc.dram_tensor("shT", [C, C], BF16, kind="ExternalInput")
    fwkT_d = nc.dram_tensor("fwkT", [C, F], BF16, kind="ExternalInput")
    fwrT_d = nc.dram_tensor("fwrT", [C, C], BF16, kind="ExternalInput")
    fwvT_d = nc.dram_tensor("fwvT", [F, C], BF16, kind="ExternalInput")
    cvec_d = nc.dram_tensor("cvec", [P, NSLOT, CT], F32, kind="ExternalInput")
    fvec_d = nc.dram_tensor("fvec", [P, 2, FT], F32, kind="ExternalInput")
    srow_d = nc.dram_tensor("srow", [1, C], BF16, kind="ExternalInput")
    out_d = nc.dram_tensor("out", [T, C], F32, kind="ExternalOutput")

    # DRAM scratch
    hnTd = nc.dram_tensor("hnTd", [C, T], BF16)
    rwTd = nc.dram_tensor("rwTd", [C, T], BF16)
    srTd = nc.dram_tensor("srTd", [C, T], BF16)
    kk2d = nc.dram_tensor("kk2d", [F, T], BF16)
    o1d = nc.dram_tensor("o1d", [T, C], F32)

    with tile.TileContext(nc) as tc:
        with tc.tile_pool(name="pool", bufs=1) as pl, \
             tc.tile_pool(name="psum", bufs=2, space="PSUM") as pp:

            # ---- constants ----
            cv = pl.tile([P, NSLOT, CT], F32, tag="cv")
            nc.sync.dma_start(out=cv, in_=cvec_d[:, :, :])
            fv = pl.tile([P, 2, FT], F32, tag="fv")
            nc.sync.dma_start(out=fv, in_=fvec_d[:, :, :])
            srow_bc = pl.tile([P, C], BF16, tag="srow")
            s_ap = srow_d[0:1, :]
            nc.sync.dma_start(out=srow_bc, in_=bass.AP(
                tensor=s_ap.tensor, offset=s_ap.offset, ap=[[0, P], s_ap.ap[1]]))

            def cvc(slot, ci):
                return cv[:, slot, ci:ci + 1]

            musd = pl.tile([P, 2, TT], F32, tag="musd")
            carA = pl.tile([P, CT], F32, tag="carA")
            carB = pl.tile([P, CT], F32, tag="carB")
            hcar = pl.tile([P, CT, 1], BF16, tag="hcar")
            gcar = pl.tile([P, CT, 1], BF16, tag="gcar")
            epst = pl.tile([P, 1], F32, tag="epst")
            nc.vector.memset(epst, EPS)

            # ---- weights: phase 1 ----
            wk_sb = pl.tile([P, CT, C], BF16, tag="w2m", bufs=3)
            nc.sync.dma_start(out=wk_sb, in_=wkT_d[:, :].rearrange("(ci p) co -> p ci co", p=P))
            wv_sb = pl.tile([P, CT, C], BF16, tag="w2m", bufs=3)
            nc.sync.dma_start(out=wv_sb, in_=wvT_d[:, :].rearrange("(ci p) co -> p ci co", p=P))
            wr_sb = pl.tile([P, CT, C], BF16, tag="w2m", bufs=3)
            nc.sync.dma_start(out=wr_sb, in_=wrT_d[:, :].rearrange("(ci p) co -> p ci co", p=P))

            # ================= Phase TM =================
            hn_prev = None
            for j in range(NJ):
                hnc = pl.tile([P, CT, 513], BF16, tag="hnT", bufs=2, name=f"hnc{j}")
                if j == 0:
                    nc.vector.memset(hnc[:, :, 0:1], 0.0)
                else:
                    nc.vector.tensor_copy(hnc[:, :, 0:1], hcar[:, :, :])
                for tl in range(4):
                    tt = 4 * j + tl
                    xt_ = pl.tile([P, C], F32, tag="xin", bufs=2, name=f"xt{tt}")
                    nc.sync.dma_start(out=xt_, in_=x_d[tt * P:(tt + 1) * P, :])
                    st_ = pl.tile([P, 2, 6], F32, tag="st", bufs=2, name=f"st{tt}")
                    nc.vector.bn_stats(out=st_[:, 0, :], in_=xt_[:, 0:512])
                    nc.vector.bn_stats(out=st_[:, 1, :], in_=xt_[:, 512:1024])
                    mv_ = pl.tile([P, 2], F32, tag="mv", bufs=2, name=f"mv{tt}")
                    nc.vector.bn_aggr(out=mv_, in_=st_)
                    nc.vector.tensor_copy(musd[:, 0, tt:tt + 1], mv_[:, 0:1])
                    nc.scalar.activation(musd[:, 1, tt:tt + 1], mv_[:, 1:2], ACT.Sqrt, bias=epst)
                    rstd = pl.tile([P, 1], F32, tag="rstd", bufs=2, name=f"rstd{tt}")
                    nc.vector.reciprocal(rstd, musd[:, 1, tt:tt + 1])
                    hnb = pl.tile([P, C], BF16, tag="hnn", bufs=2, name=f"hnb{tt}")
                    nc.vector.tensor_scalar(hnb, xt_, mv_[:, 0:1], rstd, AL.subtract, AL.mult)
                    for ci in range(CT):
                        trs = pl.tile([P, P], BF16, tag="trs", bufs=3, name=f"trs{tt}_{ci}")
                        nc.sync.dma_start(out=trs, in_=hnb[:, ci * P:(ci + 1) * P], transpose=True)
                        nc.gpsimd.tensor_copy(hnc[:, ci, 1 + tl * P:1 + (tl + 1) * P], trs)
                # carry out last col; spill chunk
                nc.vector.tensor_copy(hcar[:, :, :], hnc[:, :, 512:513])
                nc.sync.dma_start(
                    out=hnTd[:, :].rearrange("(ci p) t -> p ci t", p=P)[:, :, j * 512:(j + 1) * 512],
                    in_=hnc[:, :, 1:513])
                # mixes
                xk_ = pl.tile([P, CT, 512], BF16, tag="xk", bufs=1, name=f"xk{j}")
                xv_ = pl.tile([P, CT, 512], BF16, tag="xv", bufs=1, name=f"xv{j}")
                xr_ = pl.tile([P, CT, 512], BF16, tag="xr", bufs=1, name=f"xr{j}")
                for ci in range(CT):
                    d_ = pl.tile([P, 512], BF16, tag="mixd", bufs=2, name=f"d{j}_{ci}")
                    nc.vector.tensor_sub(d_, hnc[:, ci, 1:513], hnc[:, ci, 0:512])
                    nc.vector.scalar_tensor_tensor(xk_[:, ci, :], d_, cvc(S_MK, ci), hnc[:, ci, 0:512], AL.mult, AL.add)
                    nc.vector.scalar_tensor_tensor(xv_[:, ci, :], d_, cvc(S_MV, ci), hnc[:, ci, 0:512], AL.mult, AL.add)
                    nc.vector.scalar_tensor_tensor(xr_[:, ci, :], d_, cvc(S_MR, ci), hnc[:, ci, 0:512], AL.mult, AL.add)
                # k/v/r matmuls + WKV
                rw_ = pl.tile([P, CT, 512], BF16, tag="rw", name=f"rw{j}", bufs=1)
                for co in range(CT):
                    pk_ = pp.tile([P, 512], F32, tag="p0", name=f"pk{j}_{co}")
                    pv_ = pp.tile([P, 512], F32, tag="p1", name=f"pv{j}_{co}")
                    pr_ = pp.tile([P, 512], F32, tag="p2", name=f"pr{j}_{co}")
                    for ci in range(CT):
                        nc.tensor.matmul(pk_, wk_sb[:, ci, co * P:(co + 1) * P], xk_[:, ci, :],
                                         start=(ci == 0), stop=(ci == CT - 1))
                    for ci in range(CT):
                        nc.tensor.matmul(pv_, wv_sb[:, ci, co * P:(co + 1) * P], xv_[:, ci, :],
                                         start=(ci == 0), stop=(ci == CT - 1))
                    for ci in range(CT):
                        nc.tensor.matmul(pr_, wr_sb[:, ci, co * P:(co + 1) * P], xr_[:, ci, :],
                                         start=(ci == 0), stop=(ci == CT - 1))
                    if j == 0:
                        nc.vector.tensor_scalar_add(pk_[:, 0:1], pk_[:, 0:1], cvc(S_FK, co))
                        nc.vector.tensor_scalar_add(pv_[:, 0:1], pv_[:, 0:1], cvc(S_FV, co))
                        nc.vector.tensor_scalar_add(pr_[:, 0:1], pr_[:, 0:1], cvc(S_FR, co))
                    for h in range(2):
                        sl = slice(h * 256, (h + 1) * 256)
                        ek_ = pl.tile([P, 256], F32, tag="ek", bufs=2, name=f"ek{j}_{co}_{h}")
                        nc.scalar.activation(ek_, pk_[:, sl], ACT.Exp, bias=cvc(S_KB, co))
                        ekv_ = pl.tile([P, 256], F32, tag="ekv", bufs=2, name=f"ekv{j}_{co}_{h}")
                        nc.vector.scalar_tensor_tensor(ekv_, pv_[:, sl], cvc(S_VB, co), ek_, AL.add, AL.mult)
                        a_ = pl.tile([P, 257], F32, tag="a", bufs=2, name=f"a{j}_{co}_{h}")
                        b_ = pl.tile([P, 257], F32, tag="b", bufs=2, name=f"b{j}_{co}_{h}")
                        if j == 0 and h == 0:
                            nc.vector.memset(a_[:, 0:1], 0.0)
                            nc.vector.memset(b_[:, 0:1], 0.0)
                        else:
                            nc.vector.tensor_copy(a_[:, 0:1], carA[:, co:co + 1])
                            nc.vector.tensor_copy(b_[:, 0:1], carB[:, co:co + 1])
                        lam_bc = _bcast_free(cvc(S_LAM, co), 256)
                        nc.vector.tensor_tensor_scan(a_[:, 1:257], lam_bc, ekv_, a_[:, 0:1], AL.mult, AL.add)
                        nc.vector.tensor_tensor_scan(b_[:, 1:257], lam_bc, ek_, b_[:, 0:1], AL.mult, AL.add)
                        nc.vector.tensor_copy(carA[:, co:co + 1], a_[:, 256:257])
                        nc.vector.tensor_copy(carB[:, co:co + 1], b_[:, 256:257])
                        # num (in-place on ekv_), den (in-place on ek_)
                        nc.vector.scalar_tensor_tensor(ekv_, ekv_, cvc(S_EU, co), a_[:, 0:256], AL.mult, AL.add)
                        nc.vector.scalar_tensor_tensor(ek_, ek_, cvc(S_EU, co), b_[:, 0:256], AL.mult, AL.add)
                        nc.vector.reciprocal_approx_fast(out=ek_, in_=ek_)
                        nc.vector.tensor_mul(ekv_, ekv_, ek_)
                        srt_ = pl.tile([P, 256], F32, tag="srt", bufs=2, name=f"srt{j}_{co}_{h}")
                        nc.scalar.activation(srt_, pr_[:, sl], ACT.Sigmoid, bias=cvc(S_RB, co))
                        nc.vector.tensor_mul(rw_[:, co, sl], ekv_, srt_)
                nc.sync.dma_start(
                    out=rwTd[:, :].rearrange("(ci p) t -> p ci t", p=P)[:, :, j * 512:(j + 1) * 512],
                    in_=rw_)

            # ---- weights: phase 2 (reuse w2m slots) ----
            wo_sb = pl.tile([P, CT, C], BF16, tag="w2m", bufs=3)
            nc.sync.dma_start(out=wo_sb, in_=woT_d[:, :].rearrange("(ci p) co -> p ci co", p=P))
            sh_sb = pl.tile([P, CT, C], BF16, tag="w2m", bufs=3)
            nc.sync.dma_start(out=sh_sb, in_=shT_d[:, :].rearrange("(ci p) co -> p ci co", p=P))
            fwr_sb = pl.tile([P, CT, C], BF16, tag="w2m", bufs=3)
            nc.sync.dma_start(out=fwr_sb, in_=fwrT_d[:, :].rearrange("(ci p) co -> p ci co", p=P))
            fwk_sb = pl.tile([P, CT, F], BF16, tag="w8")
            nc.sync.dma_start(out=fwk_sb, in_=fwkT_d[:, :].rearrange("(ci p) fo -> p ci fo", p=P))

            # ================= Phase ATT + FFN-A =================
            for j in range(NJ):
                rwin = pl.tile([P, CT, 512], BF16, tag="rw", name=f"rwi{j}", bufs=1)
                nc.sync.dma_start(
                    in_=rwTd[:, :].rearrange("(ci p) t -> p ci t", p=P)[:, :, j * 512:(j + 1) * 512],
                    out=rwin)
                hnin = pl.tile([P, CT, 512], BF16, tag="hnT", bufs=2, name=f"hni{j}")
                nc.sync.dma_start(
                    in_=hnTd[:, :].rearrange("(ci p) t -> p ci t", p=P)[:, :, j * 512:(j + 1) * 512],
                    out=hnin)
                gnc = pl.tile([P, CT, 513], BF16, tag="hnT", bufs=2, name=f"gnc{j}")
                if j == 0:
                    nc.vector.memset(gnc[:, :, 0:1], 0.0)
                else:
                    nc.vector.tensor_copy(gnc[:, :, 0:1], gcar[:, :, :])
                for tl in range(4):
                    tt = 4 * j + tl
                    o1 = pl.tile([P, C], F32, tag="xin", bufs=2, name=f"o1_{tt}")
                    for nco in range(NC2):
                        ps_ = pp.tile([P, 512], F32, tag="p0", name=f"ps{tt}_{nco}")
                        pw_ = pp.tile([P, 512], F32, tag="p1", name=f"pw{tt}_{nco}")
                        for ci in range(CT):
                            nc.tensor.matmul(ps_, hnin[:, ci, tl * P:(tl + 1) * P],
                                             sh_sb[:, ci, nco * 512:(nco + 1) * 512],
                                             start=(ci == 0), stop=(ci == CT - 1))
                        for ci in range(CT):
                            nc.tensor.matmul(pw_, rwin[:, ci, tl * P:(tl + 1) * P],
                                             wo_sb[:, ci, nco * 512:(nco + 1) * 512],
                                             start=(ci == 0), stop=(ci == CT - 1))
                        tmp1 = pl.tile([P, 512], F32, tag="at1", bufs=2, name=f"at{tt}_{nco}")
                        nc.scalar.mul(tmp1, ps_, musd[:, 1, tt:tt + 1])
                        nc.vector.scalar_tensor_tensor(pw_, srow_bc[:, nco * 512:(nco + 1) * 512],
                                                       musd[:, 0, tt:tt + 1], pw_, AL.mult, AL.add)
                        nc.vector.tensor_add(o1[:, nco * 512:(nco + 1) * 512], tmp1, pw_)
                    nc.sync.dma_start(out=o1d[tt * P:(tt + 1) * P, :], in_=o1)
                    # ln2
                    st2 = pl.tile([P, 2, 6], F32, tag="st", bufs=2, name=f"st2_{tt}")
                    nc.vector.bn_stats(out=st2[:, 0, :], in_=o1[:, 0:512])
                    nc.vector.bn_stats(out=st2[:, 1, :], in_=o1[:, 512:1024])
                    mv2 = pl.tile([P, 2], F32, tag="mv", bufs=2, name=f"mv2_{tt}")
                    nc.vector.bn_aggr(out=mv2, in_=st2)
                    sd2 = pl.tile([P, 1], F32, tag="sd2", bufs=2, name=f"sd2_{tt}")
                    nc.scalar.activation(sd2, mv2[:, 1:2], ACT.Sqrt, bias=epst)
                    rstd2 = pl.tile([P, 1], F32, tag="rstd", bufs=2, name=f"rstd2_{tt}")
                    nc.vector.reciprocal(rstd2, sd2)
                    gnb = pl.tile([P, C], BF16, tag="hnn", bufs=2, name=f"gnb{tt}")
                    nc.vector.tensor_scalar(gnb, o1, mv2[:, 0:1], rstd2, AL.subtract, AL.mult)
                    for ci in range(CT):
                        trs2 = pl.tile([P, P], BF16, tag="trs", bufs=3, name=f"trs2_{tt}_{ci}")
                        nc.sync.dma_start(out=trs2, in_=gnb[:, ci * P:(ci + 1) * P], transpose=True)
                        nc.gpsimd.tensor_copy(gnc[:, ci, 1 + tl * P:1 + (tl + 1) * P], trs2)
                nc.vector.tensor_copy(gcar[:, :, :], gnc[:, :, 512:513])
                # ffn mixes
                gk_ = pl.tile([P, CT, 512], BF16, tag="xk", name=f"gk{j}", bufs=1)
                gr_ = pl.tile([P, CT, 512], BF16, tag="xv", bufs=1, name=f"gr{j}")
                for ci in range(CT):
                    d2 = pl.tile([P, 512], BF16, tag="mixd", bufs=2, name=f"d2_{j}_{ci}")
                    nc.vector.tensor_sub(d2, gnc[:, ci, 1:513], gnc[:, ci, 0:512])
                    nc.vector.scalar_tensor_tensor(gk_[:, ci, :], d2, cvc(S_FFK, ci), gnc[:, ci, 0:512], AL.mult, AL.add)
                    nc.vector.scalar_tensor_tensor(gr_[:, ci, :], d2, cvc(S_FFR, ci), gnc[:, ci, 0:512], AL.mult, AL.add)
                # kk = relu(fwk @ gk)^2 -> spill
                for ft in range(FT):
                    pkk = pp.tile([P, 512], F32, tag="p2", name=f"pkk{j}_{ft}")
                    for ci in range(CT):
                        nc.tensor.matmul(pkk, fwk_sb[:, ci, ft * P:(ft + 1) * P], gk_[:, ci, :],
                                         start=(ci == 0), stop=(ci == CT - 1))
                    if j == 0:
                        nc.vector.tensor_scalar_add(pkk[:, 0:1], pkk[:, 0:1], fv[:, 1, ft:ft + 1])
                    kr = pl.tile([P, 512], F32, tag="kr", bufs=2, name=f"kr{j}_{ft}")
                    nc.scalar.activation(kr, pkk, ACT.Relu, bias=fv[:, 0, ft:ft + 1])
                    k2 = pl.tile([P, 512], BF16, tag="k2", bufs=2, name=f"k2_{j}_{ft}")
                    nc.vector.tensor_mul(k2, kr, kr)
                    nc.sync.dma_start(out=kk2d[ft * P:(ft + 1) * P, j * 512:(j + 1) * 512], in_=k2)
                # rr -> sigmoid -> srT spill
                for co in range(CT):
                    prr = pp.tile([P, 512], F32, tag="p3", name=f"prr{j}_{co}")
                    for ci in range(CT):
                        nc.tensor.matmul(prr, fwr_sb[:, ci, co * P:(co + 1) * P], gr_[:, ci, :],
                                         start=(ci == 0), stop=(ci == CT - 1))
                    if j == 0:
                        nc.vector.tensor_scalar_add(prr[:, 0:1], prr[:, 0:1], cvc(S_FRR, co))
                    srtc = pl.tile([P, 512], BF16, tag="sc", bufs=2, name=f"sc{j}_{co}")
                    nc.scalar.activation(srtc, prr, ACT.Sigmoid, bias=cvc(S_RRB, co))
                    nc.sync.dma_start(out=srTd[co * P:(co + 1) * P, j * 512:(j + 1) * 512], in_=srtc)

            # ---- weights: phase 3 ----
            fwv_sb = pl.tile([P, FT, C], BF16, tag="w8")
            nc.sync.dma_start(out=fwv_sb, in_=fwvT_d[:, :].rearrange("(fi p) co -> p fi co", p=P))

            # ================= Phase KV (final) =================
            for tt in range(TT):
                kc0 = pl.tile([P, 16, P], BF16, tag="xk", name=f"kc0_{tt}", bufs=1)
                nc.sync.dma_start(out=kc0, in_=kk2d[0:2048, tt * P:(tt + 1) * P].rearrange("(f p) t -> p f t", p=P))
                kc1 = pl.tile([P, 16, P], BF16, tag="xv", bufs=1, name=f"kc1_{tt}")
                nc.sync.dma_start(out=kc1, in_=kk2d[2048:4096, tt * P:(tt + 1) * P].rearrange("(f p) t -> p f t", p=P))
                o1r = pl.tile([P, C], F32, tag="xin", bufs=2, name=f"o1r{tt}")
                nc.sync.dma_start(out=o1r, in_=o1d[tt * P:(tt + 1) * P, :])
                srn = pl.tile([P, C], BF16, tag="srn", bufs=2, name=f"srn{tt}")
                for co in range(CT):
                    trs3 = pl.tile([P, P], BF16, tag="trs", bufs=3, name=f"trs3_{tt}_{co}")
                    nc.sync.dma_start(out=trs3,
                                        in_=srTd[co * P:(co + 1) * P, tt * P:(tt + 1) * P],
                                        transpose=True)
                    nc.gpsimd.tensor_copy(srn[:, co * P:(co + 1) * P], trs3)
                for nco in range(NC2):
                    pkv = pp.tile([P, 512], F32, tag="p0", name=f"pkv{tt}_{nco}")
                    for ft in range(FT):
                        lhs = kc0 if ft < 16 else kc1
                        nc.tensor.matmul(pkv, lhs[:, ft % 16, :], fwv_sb[:, ft, nco * 512:(nco + 1) * 512],
                                         start=(ft == 0), stop=(ft == FT - 1))
                    tmpv = pl.tile([P, 512], F32, tag="kvt", bufs=1, name=f"kvt{tt}_{nco}")
                    nc.vector.tensor_mul(tmpv, pkv, srn[:, nco * 512:(nco + 1) * 512])
                    nc.vector.tensor_add(o1r[:, nco * 512:(nco + 1) * 512],
                                         o1r[:, nco * 512:(nco + 1) * 512], tmpv)
                nc.sync.dma_start(out=out_d[tt * P:(tt + 1) * P, :], in_=o1r)

    nc.compile()
    return nc


_NC_CACHE = {}


def get_nc(T):
    if T not in _NC_CACHE:
        _NC_CACHE[T] = build_nc(T)
    return _NC_CACHE[T]


def host_prep(inp, T):
    """Build per-core in_maps from full inputs (float64 math on host)."""
    f8 = lambda a: np.asarray(a, np.float64)
    x = np.asarray(inp["x"], np.float32)
    w1, b1 = f8(inp["ln1_w"]), f8(inp["ln1_b"])
    w2, b2 = f8(inp["ln2_w"]), f8(inp["ln2_b"])
    Wk, Wv, Wr, Wo = f8(inp["att_Wk"]), f8(inp["att_Wv"]), f8(inp["att_Wr"]), f8(inp["att_Wo"])
    Wsh = f8(inp["short_W"])
    fWk, fWr, fWv = f8(inp["ffn_Wk"]), f8(inp["ffn_Wr"]), f8(inp["ffn_Wv"])
    mk, mvx, mr = f8(inp["att_mix_k"]), f8(inp["att_mix_v"]), f8(inp["att_mix_r"])
    fk, fr = f8(inp["ffn_mix_k"]), f8(inp["ffn_mix_r"])
    decay, first = f8(inp["att_time_decay"]), f8(inp["att_time_first"])

    def pack_c(v):
        return np.asarray(v, np.float32).reshape(CT, P).T  # [128, CT]

    lam = np.exp(-np.exp(decay))
    eu = np.exp(first)
    kbias = Wk @ b1
    vbias = Wv @ b1
    rbias = Wr @ b1
    fixk = -Wk @ ((1.0 - mk) * b1)
    fixv = -Wv @ ((1.0 - mvx) * b1)
    fixr = -Wr @ ((1.0 - mr) * b1)
    kkbias = fWk @ b2
    fixkk = -fWk @ ((1.0 - fk) * b2)
    rrbias = fWr @ b2
    fixrr = -fWr @ ((1.0 - fr) * b2)
    srow = Wsh.sum(axis=1)

    cvec = np.stack([pack_c(v) for v in
                     [lam, eu, mk, mvx, mr, kbias, vbias, rbias,
                      fixk, fixv, fixr, fk, fr, rrbias, fixrr]], axis=1)  # [128, 15, 8]
    fvec = np.stack([np.asarray(v, np.float32).reshape(FT, P).T for v in [kkbias, fixkk]],
                    axis=1)  # [128, 2, 32]

    shared = {
        "wkT": np.ascontiguousarray((Wk * w1[None, :]).T.astype(BF)),
        "wvT": np.ascontiguousarray((Wv * w1[None, :]).T.astype(BF)),
        "wrT": np.ascontiguousarray((Wr * w1[None, :]).T.astype(BF)),
        "woT": np.ascontiguousarray(Wo.T.astype(BF)),
        "shT": np.ascontiguousarray(Wsh.T.astype(BF)),
        "fwkT": np.ascontiguousarray((fWk * w2[None, :]).T.astype(BF)),
        "fwrT": np.ascontiguousarray((fWr * w2[None, :]).T.astype(BF)),
        "fwvT": np.ascontiguousarray(fWv.T.astype(BF)),
        "cvec": np.ascontiguousarray(cvec.astype(np.float32)),
        "fvec": np.ascontiguousarray(fvec.astype(np.float32)),
        "srow": np.ascontiguousarray(srow.reshape(1, C).astype(BF)),
    }
    in_maps = []
    for b in range(x.shape[0]):
        m = dict(shared)
        m["x"] = np.ascontiguousarray(x[b, :T, :])
        in_maps.append(m)
    return in_maps


def kernel(**inputs):
    T = 2048
    nc = get_nc(T)
    in_maps = host_prep(inputs, T)
    res = run_bass_kernel_spmd(nc, in_maps, core_ids=list(range(len(in_maps))))
    out = np.stack([r["out"] for r in res.results], axis=0)
    return out.astype(np.float32)

